# revision 18
# baseline (speedup 1.0000x reference)
"""ChannelRowAttention Trainium2 kernel (v2).

Full-input contract: kernel(**inputs) takes the complete (8,256,128,128) batch
plus weights, shards batch-wise across 8 NeuronCores (one image per core), and
returns the full (8,256,128,128) output.

v2 design (per core, x_img = (256,128,128)):
  HBM traffic minimized to x-in fp16 (16.8MB) + y-out fp16 (16.8MB); x stays
  resident in SBUF for both passes, attention output stays resident fp16.

  pass 1, per 4-row block:
    qk    = [Wk|Wq]^T . x_rows       one fused M=128 matmul pair (k on
            partitions 0:64, q on 64:128); ACT copy to fp16; q replicated to
            a base-0 tile via SBUF->SBUF DMA
    attT  = k^T q  (PE, K=64)        -- attention computed TRANSPOSED, so no
            PE transpose and one batched exp serves the whole block
    att_e = exp(attT) -> bf16        (one ACT instr per block)
    den   = ones^T . att_e           (PE matmul, M=1)
    inv   = 1/den                    (DVE reciprocal, fp32)
    binv  = broadcast(inv) across partitions via SBUF->SBUF DMA (free)
    vT    = x_row^T . Wv^T           (PE, per row; copies split ACT/GPSIMD)
    out_u = vT^T . att_e             (PE; unnormalized)
    out   = out_u * binv             rides the PSUM->SBUF copy (DVE STT);
            per-channel mean rides accum_out; running max on GPSIMD
  gate  = sigmoid(W2.relu(W1.avg) + W2.relu(W1.max)) via tanh trick
  pass 2, per block: final = out*(gama*gate[c]) + x -> fp16 -> DRAM
"""

import numpy as np
from contextlib import ExitStack

import concourse.bass as bass
from concourse import bacc
import concourse.tile as tile
from concourse import mybir
from concourse.bass_utils import run_bass_kernel_spmd

F32 = mybir.dt.float32
F16 = mybir.dt.float16
BF16 = mybir.dt.bfloat16

N, C, H, W = 8, 256, 128, 128
QK = 64
HID = 16          # SE hidden dim = C // 16
NCORES = 8
RB = 4            # rows per block
NBLK = H // RB    # 32
INV_HW = 1.0 / float(H * W)

AX = mybir.AxisListType
OP = mybir.AluOpType
AF = mybir.ActivationFunctionType


def _body(ctx: ExitStack, tc: "tile.TileContext", xh_d, wqk_d, wv_d,
          w1_d, w2_d, gama_d, y_d):
    nc = tc.nc

    const = ctx.enter_context(tc.tile_pool(name="const", bufs=1))
    stats = ctx.enter_context(tc.tile_pool(name="stats", bufs=1))
    xpool = ctx.enter_context(tc.tile_pool(name="xp", bufs=1))
    opool = ctx.enter_context(tc.tile_pool(name="op", bufs=1))
    work = ctx.enter_context(tc.tile_pool(name="work", bufs=3))
    finpool = ctx.enter_context(tc.tile_pool(name="fin", bufs=3))
    psQ = ctx.enter_context(tc.tile_pool(name="psQ", bufs=1, space="PSUM"))
    psV = ctx.enter_context(tc.tile_pool(name="psV", bufs=2, space="PSUM"))
    psA = ctx.enter_context(tc.tile_pool(name="psA", bufs=1, space="PSUM"))
    psD = ctx.enter_context(tc.tile_pool(name="psD", bufs=1, space="PSUM"))
    psO = ctx.enter_context(tc.tile_pool(name="psO", bufs=1, space="PSUM"))

    # ---- constants -------------------------------------------------------
    wqk_sb = const.tile([128, 2, 128], F16)
    nc.sync.dma_start(out=wqk_sb, in_=wqk_d[:, :].rearrange("(kc p) m -> p kc m", p=128))
    wv_sb = const.tile([128, 2, C], F16)
    nc.sync.dma_start(out=wv_sb, in_=wv_d[:, :].rearrange("(kc p) m -> p kc m", p=128))
    w1_sb = const.tile([128, 2, HID], F32)
    nc.sync.dma_start(out=w1_sb, in_=w1_d[:, :].rearrange("(kc p) m -> p kc m", p=128))
    w2_sb = const.tile([HID, 2, 128], F32)
    nc.sync.dma_start(out=w2_sb, in_=w2_d[:, :].rearrange("k (mc m) -> k mc m", m=128))
    gama_sb = const.tile([128, 1], F32)
    nc.sync.dma_start(out=gama_sb, in_=gama_d[:, :].to_broadcast([128, 1]))
    ones_col = const.tile([128, 1], BF16)
    nc.vector.memset(ones_col, 1.0)
    gscale = const.tile([128, 2], F32)      # gama * sigmoid(gate), filled later

    sums_acc = stats.tile([128, 2, NBLK], F32)
    nc.vector.memset(sums_acc, 0.0)
    MXCH = NBLK // 8            # max-reduce chunk count (every 8 blocks)
    mx_acc = stats.tile([128, MXCH, 2], F16)

    # ---- resident x: all DMAs issued up front ---------------------------
    x_tiles = []
    for b in range(NBLK):
        h0 = b * RB
        xb = xpool.tile([128, 2, RB, W], F16, tag=f"x{b}")
        nc.sync.dma_start(
            out=xb,
            in_=xh_d[:, h0:h0 + RB, :].rearrange("(kc p) h w -> p kc h w", p=128),
        )
        x_tiles.append(xb)

    # single big resident attention-output tile (fine-grained dep tracking is
    # not needed: pass-2 and the max reduces only read after prior writes)
    ob_all = opool.tile([128, 2, NBLK, RB, W], F16, tag="ob")

    # ---- pass 1 ----------------------------------------------------------
    for b in range(NBLK):
        xb = x_tiles[b]

        # fused q/k projection: k on partitions 0:64, q on 64:128
        qk_ps = psQ.tile([128, RB, W], F32, tag="qk")
        for kc in (0, 1):
            nc.tensor.matmul(
                out=qk_ps.rearrange("p r w -> p (r w)"),
                lhsT=wqk_sb[:, kc, :],
                rhs=xb[:, kc, :, :].rearrange("p r w -> p (r w)"),
                start=(kc == 0), stop=(kc == 1),
            )
        qk_sb = work.tile([128, RB, W], F16, tag="qk_sb")
        nc.scalar.copy(out=qk_sb, in_=qk_ps)
        # replicate q down to base partition 0 (matmul operands share base)
        qq_sb = work.tile([64, RB, W], F16, tag="qq_sb")
        nc.sync.dma_start(out=qq_sb, in_=qk_sb[64:128, :, :])

        # v^T per row (w on partitions, c on free)
        vt_sb = work.tile([128, RB, C], BF16, tag="vt_sb")
        for rp in range(RB // 2):
            vt_ps = psV.tile([128, 2, C], F32, tag="vt")
            for rr in (0, 1):
                r = rp * 2 + rr
                for kc in (0, 1):
                    nc.tensor.matmul(
                        out=vt_ps[:, rr, :],
                        lhsT=xb[:, kc, r, :],
                        rhs=wv_sb[:, kc, :],
                        start=(kc == 0), stop=(kc == 1),
                    )
            nc.scalar.copy(out=vt_sb[:, rp * 2:rp * 2 + 2, :], in_=vt_ps)

        # transposed attention: attT[j, i] = sum_d k[d, j] q[d, i]
        attT_ps = psA.tile([128, RB, W], F32, tag="attT")
        for r in range(RB):
            nc.tensor.matmul(
                out=attT_ps[:, r, :],
                lhsT=qk_sb[0:64, r, :],
                rhs=qq_sb[:, r, :],
                start=True, stop=True,
            )
        # batched exp for the whole block; no max-subtraction (|score|<~40)
        att_e = work.tile([128, RB, W], BF16, tag="att_e")
        nc.scalar.activation(out=att_e, in_=attT_ps, func=AF.Exp)

        # denominator via ones-matmul (cross-partition sum), then 1/x on DVE
        den_ps = psD.tile([1, RB, W], F32, tag="den")
        nc.tensor.matmul(
            out=den_ps.rearrange("p r w -> p (r w)"),
            lhsT=ones_col,
            rhs=att_e.rearrange("p r w -> p (r w)"),
            start=True, stop=True,
        )
        inv_sb = work.tile([1, RB, W], F32, tag="inv")
        nc.vector.reciprocal(out=inv_sb, in_=den_ps)
        # broadcast 1/den across partitions on GPSIMD (SBUF->SBUF)
        binv_sb = work.tile([128, RB, W], F32, tag="binv")
        nc.gpsimd.partition_broadcast(binv_sb[:, :, :], inv_sb[:, :, :])

        # unnormalized out; normalization + mean stat ride the PSUM->SBUF copy
        out_ps = psO.tile([128, 2, RB, W], F32, tag="out")
        for r in range(RB):
            for ch in (0, 1):
                nc.tensor.matmul(
                    out=out_ps[:, ch, r, :],
                    lhsT=vt_sb[:, r, 128 * ch:128 * (ch + 1)],
                    rhs=att_e[:, r, :],
                    start=True, stop=True,
                )

        for ch in (0, 1):
            nc.vector.scalar_tensor_tensor(
                out=ob_all[:, ch, b], in0=out_ps[:, ch], scalar=1.0,
                in1=binv_sb, op0=OP.mult, op1=OP.mult,
                accum_out=sums_acc[:, ch, b:b + 1])
        # chunked max stat: one 2x-mode DVE reduce per 8 blocks
        if b % 8 == 7:
            k = b // 8
            nc.vector.tensor_reduce(
                out=mx_acc[:, k, :], in_=ob_all[:, :, b - 7:b + 1, :, :],
                axis=AX.XYZ, op=OP.max)

    # ---- gate ------------------------------------------------------------
    sums = stats.tile([128, 2], F32)
    nc.vector.tensor_reduce(out=sums, in_=sums_acc, axis=AX.X, op=OP.add)

    mx = stats.tile([128, 2], F32)
    nc.vector.tensor_reduce(
        out=mx, in_=mx_acc.rearrange("p k c -> p c k"), axis=AX.X, op=OP.max)

    mlp_in = stats.tile([128, 2, 2], F32)
    nc.vector.tensor_scalar_mul(out=mlp_in[:, :, 0], in0=sums, scalar1=INV_HW)
    nc.vector.tensor_copy(out=mlp_in[:, :, 1], in_=mx)

    h_ps = psD.tile([HID, 2], F32, tag="den")
    for kc in (0, 1):
        nc.tensor.matmul(
            out=h_ps,
            lhsT=w1_sb[:, kc, :],
            rhs=mlp_in[:, kc, :],
            start=(kc == 0), stop=(kc == 1),
        )
    hr = stats.tile([HID, 2], F32)
    nc.vector.tensor_scalar_max(out=hr, in0=h_ps, scalar1=0.0)
    g_ps = psQ.tile([128, 2, 2], F32, tag="qk")
    for mc in (0, 1):
        nc.tensor.matmul(
            out=g_ps[:, mc, :],
            lhsT=w2_sb[:, mc, :],
            rhs=hr,
            start=True, stop=True,
        )
    zt = stats.tile([128, 2], F32)
    nc.vector.tensor_reduce(out=zt, in_=g_ps, axis=AX.X, op=OP.add)
    th = stats.tile([128, 2], F32)
    nc.scalar.activation(out=th, in_=zt, func=AF.Tanh, scale=0.5)
    u = stats.tile([128, 2], F32)
    nc.vector.tensor_scalar_add(out=u, in0=th, scalar1=1.0)
    # gscale = gama * sigmoid(z) = gama * 0.5 * (1 + tanh(z/2))
    nc.vector.tensor_scalar(
        out=gscale, in0=u, scalar1=gama_sb, scalar2=0.5, op0=OP.mult, op1=OP.mult)

    # ---- pass 2: final = out*gscale[c] + x ------------------------------
    for b in range(NBLK):
        h0 = b * RB
        xb = x_tiles[b]
        fin = finpool.tile([128, 2, RB, W], F16, tag="fin")
        for kc in (0, 1):
            nc.vector.scalar_tensor_tensor(
                out=fin[:, kc], in0=ob_all[:, kc, b],
                scalar=gscale[:, kc:kc + 1],
                in1=xb[:, kc], op0=OP.mult, op1=OP.add)
        nc.sync.dma_start(
            out=y_d[:, h0:h0 + RB, :].rearrange("(kc p) h w -> p kc h w", p=128),
            in_=fin,
        )


def build_nc() -> bass.Bass:
    nc = bacc.Bacc()
    xh_d = nc.dram_tensor("xh", [C, H, W], F16, kind="ExternalInput")
    wqk_d = nc.dram_tensor("wqkT", [C, 128], F16, kind="ExternalInput")
    wv_d = nc.dram_tensor("wvT", [C, C], F16, kind="ExternalInput")
    w1_d = nc.dram_tensor("w1T", [C, HID], F32, kind="ExternalInput")
    w2_d = nc.dram_tensor("w2T", [HID, C], F32, kind="ExternalInput")
    gama_d = nc.dram_tensor("gama", [1, 1], F32, kind="ExternalInput")
    y_d = nc.dram_tensor("out", [C, H, W], F16, kind="ExternalOutput")

    with tile.TileContext(nc) as tc:
        with ExitStack() as ctx:
            _body(ctx, tc, xh_d[:, :, :], wqk_d[:, :],
                  wv_d[:, :], w1_d[:, :], w2_d[:, :], gama_d[:, :],
                  y_d[:, :, :])
    nc.compile()
    return nc


_NC_CACHE = {}


def _get_nc():
    if "nc" not in _NC_CACHE:
        _NC_CACHE["nc"] = build_nc()
    return _NC_CACHE["nc"]


def _make_in_maps(x, Wq, Wk, Wv, W1, W2, gama):
    wqkT = np.ascontiguousarray(
        np.concatenate([Wk, Wq], axis=0).T.astype(np.float16))
    wvT = np.ascontiguousarray(Wv.T.astype(np.float16))
    w1T = np.ascontiguousarray(W1.T.astype(np.float32))
    w2T = np.ascontiguousarray(W2.T.astype(np.float32))
    g = np.asarray(gama, dtype=np.float32).reshape(1, 1)
    maps = []
    for i in range(NCORES):
        maps.append({
            "xh": np.ascontiguousarray(x[i].astype(np.float16)),
            "wqkT": wqkT, "wvT": wvT, "w1T": w1T, "w2T": w2T, "gama": g,
        })
    return maps


def run(x, Wq, Wk, Wv, W1, W2, gama, trace=False):
    nc = _get_nc()
    in_maps = _make_in_maps(x, Wq, Wk, Wv, W1, W2, gama)
    res = run_bass_kernel_spmd(nc, in_maps, core_ids=list(range(NCORES)),
                               trace=trace)
    y = np.stack([res.results[i]["out"] for i in range(NCORES)], axis=0)
    return y, res


def kernel(x, Wq, Wk, Wv, W1, W2, gama):
    x = np.asarray(x); Wq = np.asarray(Wq); Wk = np.asarray(Wk)
    Wv = np.asarray(Wv); W1 = np.asarray(W1); W2 = np.asarray(W2)
    gama = np.asarray(gama)
    y, _ = run(x, Wq, Wk, Wv, W1, W2, gama, trace=False)
    return y.astype(np.float32)


# revision 20
# speedup vs baseline: 1.4108x; 1.4108x over previous
"""ChannelRowAttention Trainium2 kernel (v2).

Full-input contract: kernel(**inputs) takes the complete (8,256,128,128) batch
plus weights, shards batch-wise across 8 NeuronCores (one image per core), and
returns the full (8,256,128,128) output.

v2 design (per core, x_img = (256,128,128)):
  HBM traffic minimized to x-in fp16 (16.8MB) + y-out fp16 (16.8MB); x stays
  resident in SBUF for both passes, attention output stays resident fp16.

  pass 1, per 4-row block:
    qk    = [Wk|Wq]^T . x_rows       one fused M=128 matmul pair (k on
            partitions 0:64, q on 64:128); ACT copy to fp16; q replicated to
            a base-0 tile via SBUF->SBUF DMA
    attT  = k^T q  (PE, K=64)        -- attention computed TRANSPOSED, so no
            PE transpose and one batched exp serves the whole block
    att_e = exp(attT) -> bf16        (one ACT instr per block)
    den   = ones^T . att_e           (PE matmul, M=1)
    inv   = 1/den                    (DVE reciprocal, fp32)
    binv  = broadcast(inv) across partitions via SBUF->SBUF DMA (free)
    vT    = x_row^T . Wv^T           (PE, per row; copies split ACT/GPSIMD)
    out_u = vT^T . att_e             (PE; unnormalized)
    out   = out_u * binv             rides the PSUM->SBUF copy (DVE STT);
            per-channel mean rides accum_out; running max on GPSIMD
  gate  = sigmoid(W2.relu(W1.avg) + W2.relu(W1.max)) via tanh trick
  pass 2, per block: final = out*(gama*gate[c]) + x -> fp16 -> DRAM
"""

import numpy as np
from contextlib import ExitStack

import concourse.bass as bass
from concourse import bacc
import concourse.tile as tile
from concourse import mybir
from concourse.bass_utils import run_bass_kernel_spmd

F32 = mybir.dt.float32
F16 = mybir.dt.float16
BF16 = mybir.dt.bfloat16

N, C, H, W = 8, 256, 128, 128
QK = 64
HID = 16          # SE hidden dim = C // 16
NCORES = 8
RB = 4            # rows per block
NBLK = H // RB    # 32
INV_HW = 1.0 / float(H * W)

AX = mybir.AxisListType
OP = mybir.AluOpType
AF = mybir.ActivationFunctionType


def _body(ctx: ExitStack, tc: "tile.TileContext", xh_d, wqk_d, wv_d,
          w1_d, w2_d, gama_d, y_d):
    nc = tc.nc

    const = ctx.enter_context(tc.tile_pool(name="const", bufs=1))
    stats = ctx.enter_context(tc.tile_pool(name="stats", bufs=1))
    xpool = ctx.enter_context(tc.tile_pool(name="xp", bufs=1))
    opool = ctx.enter_context(tc.tile_pool(name="op", bufs=1))
    work = ctx.enter_context(tc.tile_pool(name="work", bufs=3))
    finpool = ctx.enter_context(tc.tile_pool(name="fin", bufs=3))
    psQ = ctx.enter_context(tc.tile_pool(name="psQ", bufs=1, space="PSUM"))
    psV = ctx.enter_context(tc.tile_pool(name="psV", bufs=2, space="PSUM"))
    psA = ctx.enter_context(tc.tile_pool(name="psA", bufs=1, space="PSUM"))
    psD = ctx.enter_context(tc.tile_pool(name="psD", bufs=1, space="PSUM"))
    psO = ctx.enter_context(tc.tile_pool(name="psO", bufs=1, space="PSUM"))

    # ---- constants -------------------------------------------------------
    wqk_sb = const.tile([128, 2, 128], F16)
    nc.sync.dma_start(out=wqk_sb, in_=wqk_d[:, :].rearrange("(kc p) m -> p kc m", p=128))
    wv_sb = const.tile([128, 2, C], F16)
    nc.sync.dma_start(out=wv_sb, in_=wv_d[:, :].rearrange("(kc p) m -> p kc m", p=128))
    w1_sb = const.tile([128, 2, HID], F32)
    nc.sync.dma_start(out=w1_sb, in_=w1_d[:, :].rearrange("(kc p) m -> p kc m", p=128))
    w2_sb = const.tile([HID, 2, 128], F32)
    nc.sync.dma_start(out=w2_sb, in_=w2_d[:, :].rearrange("k (mc m) -> k mc m", m=128))
    gama_sb = const.tile([128, 1], F32)
    nc.sync.dma_start(out=gama_sb, in_=gama_d[:, :].to_broadcast([128, 1]))
    ones_col = const.tile([128, 1], BF16)
    nc.vector.memset(ones_col, 1.0)
    gscale = const.tile([128, 2], F32)      # gama * sigmoid(gate), filled later

    sums_acc = stats.tile([128, 2, NBLK], F32)
    nc.vector.memset(sums_acc, 0.0)
    MXCH = NBLK // 8            # max-reduce chunk count (every 8 blocks)
    mx_acc = stats.tile([128, MXCH, 2], F16)

    # ---- resident x: all DMAs issued up front ---------------------------
    x_tiles = []
    for b in range(NBLK):
        h0 = b * RB
        xb = xpool.tile([128, 2, RB, W], F16, tag=f"x{b}")
        nc.sync.dma_start(
            out=xb,
            in_=xh_d[:, h0:h0 + RB, :].rearrange("(kc p) h w -> p kc h w", p=128),
        )
        x_tiles.append(xb)

    # single big resident attention-output tile (fine-grained dep tracking is
    # not needed: pass-2 and the max reduces only read after prior writes)
    ob_all = opool.tile([128, 2, NBLK, RB, W], F16, tag="ob")

    # ---- pass 1 ----------------------------------------------------------
    for b in range(NBLK):
        xb = x_tiles[b]

        # fused q/k projection: k on partitions 0:64, q on 64:128
        qk_ps = psQ.tile([128, RB, W], F32, tag="qk")
        for kc in (0, 1):
            nc.tensor.matmul(
                out=qk_ps.rearrange("p r w -> p (r w)"),
                lhsT=wqk_sb[:, kc, :],
                rhs=xb[:, kc, :, :].rearrange("p r w -> p (r w)"),
                start=(kc == 0), stop=(kc == 1),
            )
        qk_sb = work.tile([128, RB, W], F16, tag="qk_sb")
        nc.scalar.copy(out=qk_sb, in_=qk_ps)
        # replicate q down to base partition 0 (matmul operands share base);
        # issue from the Scalar engine so the SP DMA queue never stalls on it
        qq_sb = work.tile([64, RB, W], F16, tag="qq_sb")
        nc.scalar.dma_start(out=qq_sb, in_=qk_sb[64:128, :, :])

        # v^T per row (w on partitions, c on free)
        vt_sb = work.tile([128, RB, C], BF16, tag="vt_sb")
        for rp in range(RB // 2):
            vt_ps = psV.tile([128, 2, C], F32, tag="vt")
            for rr in (0, 1):
                r = rp * 2 + rr
                for kc in (0, 1):
                    nc.tensor.matmul(
                        out=vt_ps[:, rr, :],
                        lhsT=xb[:, kc, r, :],
                        rhs=wv_sb[:, kc, :],
                        start=(kc == 0), stop=(kc == 1),
                    )
            nc.scalar.copy(out=vt_sb[:, rp * 2:rp * 2 + 2, :], in_=vt_ps)

        # transposed attention: attT[j, i] = sum_d k[d, j] q[d, i]
        attT_ps = psA.tile([128, RB, W], F32, tag="attT")
        for r in range(RB):
            nc.tensor.matmul(
                out=attT_ps[:, r, :],
                lhsT=qk_sb[0:64, r, :],
                rhs=qq_sb[:, r, :],
                start=True, stop=True,
            )
        # batched exp for the whole block; no max-subtraction (|score|<~40)
        att_e = work.tile([128, RB, W], BF16, tag="att_e")
        nc.scalar.activation(out=att_e, in_=attT_ps, func=AF.Exp)

        # denominator via ones-matmul (cross-partition sum), then 1/x on DVE
        den_ps = psD.tile([1, RB, W], F32, tag="den")
        nc.tensor.matmul(
            out=den_ps.rearrange("p r w -> p (r w)"),
            lhsT=ones_col,
            rhs=att_e.rearrange("p r w -> p (r w)"),
            start=True, stop=True,
        )
        inv_sb = work.tile([1, RB, W], F32, tag="inv")
        # ~18-bit 1/x, ~5x faster than the exact InstReciprocal (which
        # measured 3.3us per 512-element call on hardware)
        nc.vector.reciprocal_approx_fast(out=inv_sb, in_=den_ps)
        # broadcast 1/den across partitions on GPSIMD (SBUF->SBUF)
        binv_sb = work.tile([128, RB, W], F32, tag="binv")
        nc.gpsimd.partition_broadcast(binv_sb[:, :, :], inv_sb[:, :, :])

        # unnormalized out; normalization + mean stat ride the PSUM->SBUF copy
        out_ps = psO.tile([128, 2, RB, W], F32, tag="out")
        for r in range(RB):
            for ch in (0, 1):
                nc.tensor.matmul(
                    out=out_ps[:, ch, r, :],
                    lhsT=vt_sb[:, r, 128 * ch:128 * (ch + 1)],
                    rhs=att_e[:, r, :],
                    start=True, stop=True,
                )

        for ch in (0, 1):
            nc.vector.scalar_tensor_tensor(
                out=ob_all[:, ch, b], in0=out_ps[:, ch], scalar=1.0,
                in1=binv_sb, op0=OP.mult, op1=OP.mult,
                accum_out=sums_acc[:, ch, b:b + 1])
        # chunked max stat: one 2x-mode DVE reduce per 8 blocks
        if b % 8 == 7:
            k = b // 8
            nc.vector.tensor_reduce(
                out=mx_acc[:, k, :], in_=ob_all[:, :, b - 7:b + 1, :, :],
                axis=AX.XYZ, op=OP.max)

    # ---- gate ------------------------------------------------------------
    sums = stats.tile([128, 2], F32)
    nc.vector.tensor_reduce(out=sums, in_=sums_acc, axis=AX.X, op=OP.add)

    mx = stats.tile([128, 2], F32)
    nc.vector.tensor_reduce(
        out=mx, in_=mx_acc.rearrange("p k c -> p c k"), axis=AX.X, op=OP.max)

    mlp_in = stats.tile([128, 2, 2], F32)
    nc.vector.tensor_scalar_mul(out=mlp_in[:, :, 0], in0=sums, scalar1=INV_HW)
    nc.vector.tensor_copy(out=mlp_in[:, :, 1], in_=mx)

    h_ps = psD.tile([HID, 2], F32, tag="den")
    for kc in (0, 1):
        nc.tensor.matmul(
            out=h_ps,
            lhsT=w1_sb[:, kc, :],
            rhs=mlp_in[:, kc, :],
            start=(kc == 0), stop=(kc == 1),
        )
    hr = stats.tile([HID, 2], F32)
    nc.vector.tensor_scalar_max(out=hr, in0=h_ps, scalar1=0.0)
    g_ps = psQ.tile([128, 2, 2], F32, tag="qk")
    for mc in (0, 1):
        nc.tensor.matmul(
            out=g_ps[:, mc, :],
            lhsT=w2_sb[:, mc, :],
            rhs=hr,
            start=True, stop=True,
        )
    zt = stats.tile([128, 2], F32)
    nc.vector.tensor_reduce(out=zt, in_=g_ps, axis=AX.X, op=OP.add)
    th = stats.tile([128, 2], F32)
    nc.scalar.activation(out=th, in_=zt, func=AF.Tanh, scale=0.5)
    u = stats.tile([128, 2], F32)
    nc.vector.tensor_scalar_add(out=u, in0=th, scalar1=1.0)
    # gscale = gama * sigmoid(z) = gama * 0.5 * (1 + tanh(z/2))
    nc.vector.tensor_scalar(
        out=gscale, in0=u, scalar1=gama_sb, scalar2=0.5, op0=OP.mult, op1=OP.mult)

    # ---- pass 2: final = out*gscale[c] + x ------------------------------
    for b in range(NBLK):
        h0 = b * RB
        xb = x_tiles[b]
        fin = finpool.tile([128, 2, RB, W], F16, tag="fin")
        for kc in (0, 1):
            nc.vector.scalar_tensor_tensor(
                out=fin[:, kc], in0=ob_all[:, kc, b],
                scalar=gscale[:, kc:kc + 1],
                in1=xb[:, kc], op0=OP.mult, op1=OP.add)
        nc.sync.dma_start(
            out=y_d[:, h0:h0 + RB, :].rearrange("(kc p) h w -> p kc h w", p=128),
            in_=fin,
        )


def build_nc() -> bass.Bass:
    nc = bacc.Bacc()
    xh_d = nc.dram_tensor("xh", [C, H, W], F16, kind="ExternalInput")
    wqk_d = nc.dram_tensor("wqkT", [C, 128], F16, kind="ExternalInput")
    wv_d = nc.dram_tensor("wvT", [C, C], F16, kind="ExternalInput")
    w1_d = nc.dram_tensor("w1T", [C, HID], F32, kind="ExternalInput")
    w2_d = nc.dram_tensor("w2T", [HID, C], F32, kind="ExternalInput")
    gama_d = nc.dram_tensor("gama", [1, 1], F32, kind="ExternalInput")
    y_d = nc.dram_tensor("out", [C, H, W], F16, kind="ExternalOutput")

    with tile.TileContext(nc) as tc:
        with ExitStack() as ctx:
            _body(ctx, tc, xh_d[:, :, :], wqk_d[:, :],
                  wv_d[:, :], w1_d[:, :], w2_d[:, :], gama_d[:, :],
                  y_d[:, :, :])
    nc.compile()
    return nc


_NC_CACHE = {}


def _get_nc():
    if "nc" not in _NC_CACHE:
        _NC_CACHE["nc"] = build_nc()
    return _NC_CACHE["nc"]


def _make_in_maps(x, Wq, Wk, Wv, W1, W2, gama):
    wqkT = np.ascontiguousarray(
        np.concatenate([Wk, Wq], axis=0).T.astype(np.float16))
    wvT = np.ascontiguousarray(Wv.T.astype(np.float16))
    w1T = np.ascontiguousarray(W1.T.astype(np.float32))
    w2T = np.ascontiguousarray(W2.T.astype(np.float32))
    g = np.asarray(gama, dtype=np.float32).reshape(1, 1)
    maps = []
    for i in range(NCORES):
        maps.append({
            "xh": np.ascontiguousarray(x[i].astype(np.float16)),
            "wqkT": wqkT, "wvT": wvT, "w1T": w1T, "w2T": w2T, "gama": g,
        })
    return maps


def run(x, Wq, Wk, Wv, W1, W2, gama, trace=False):
    nc = _get_nc()
    in_maps = _make_in_maps(x, Wq, Wk, Wv, W1, W2, gama)
    res = run_bass_kernel_spmd(nc, in_maps, core_ids=list(range(NCORES)),
                               trace=trace)
    y = np.stack([res.results[i]["out"] for i in range(NCORES)], axis=0)
    return y, res


def kernel(x, Wq, Wk, Wv, W1, W2, gama):
    x = np.asarray(x); Wq = np.asarray(Wq); Wk = np.asarray(Wk)
    Wv = np.asarray(Wv); W1 = np.asarray(W1); W2 = np.asarray(W2)
    gama = np.asarray(gama)
    y, _ = run(x, Wq, Wk, Wv, W1, W2, gama, trace=False)
    return y.astype(np.float32)


# revision 24
# speedup vs baseline: 1.5131x; 1.0725x over previous
"""ChannelRowAttention Trainium2 kernel (v2).

Full-input contract: kernel(**inputs) takes the complete (8,256,128,128) batch
plus weights, shards batch-wise across 8 NeuronCores (one image per core), and
returns the full (8,256,128,128) output.

v2 design (per core, x_img = (256,128,128)):
  HBM traffic minimized to x-in fp16 (16.8MB) + y-out fp16 (16.8MB); x stays
  resident in SBUF for both passes, attention output stays resident fp16.

  pass 1, per 4-row block:
    qk    = [Wk|Wq]^T . x_rows       one fused M=128 matmul pair (k on
            partitions 0:64, q on 64:128); ACT copy to fp16; q replicated to
            a base-0 tile via SBUF->SBUF DMA
    attT  = k^T q  (PE, K=64)        -- attention computed TRANSPOSED, so no
            PE transpose and one batched exp serves the whole block
    att_e = exp(attT) -> bf16        (one ACT instr per block)
    den   = ones^T . att_e           (PE matmul, M=1)
    inv   = 1/den                    (DVE reciprocal, fp32)
    binv  = broadcast(inv) across partitions via SBUF->SBUF DMA (free)
    vT    = x_row^T . Wv^T           (PE, per row; copies split ACT/GPSIMD)
    out_u = vT^T . att_e             (PE; unnormalized)
    out   = out_u * binv             rides the PSUM->SBUF copy (DVE STT);
            per-channel mean rides accum_out; running max on GPSIMD
  gate  = sigmoid(W2.relu(W1.avg) + W2.relu(W1.max)) via tanh trick
  pass 2, per block: final = out*(gama*gate[c]) + x -> fp16 -> DRAM
"""

import numpy as np
from contextlib import ExitStack

import concourse.bass as bass
from concourse import bacc
import concourse.tile as tile
from concourse import mybir
from concourse.bass_utils import run_bass_kernel_spmd

F32 = mybir.dt.float32
F16 = mybir.dt.float16
BF16 = mybir.dt.bfloat16

N, C, H, W = 8, 256, 128, 128
QK = 64
HID = 16          # SE hidden dim = C // 16
NCORES = 8
RB = 4            # rows per block
NBLK = H // RB    # 32
INV_HW = 1.0 / float(H * W)

AX = mybir.AxisListType
OP = mybir.AluOpType
AF = mybir.ActivationFunctionType


def _body(ctx: ExitStack, tc: "tile.TileContext", xh_d, wqk_d, wv_d,
          w1_d, w2_d, gama_d, y_d):
    nc = tc.nc

    const = ctx.enter_context(tc.tile_pool(name="const", bufs=1))
    stats = ctx.enter_context(tc.tile_pool(name="stats", bufs=1))
    xpool = ctx.enter_context(tc.tile_pool(name="xp", bufs=1))
    opool = ctx.enter_context(tc.tile_pool(name="op", bufs=1))
    work = ctx.enter_context(tc.tile_pool(name="work", bufs=3))
    finpool = ctx.enter_context(tc.tile_pool(name="fin", bufs=3))
    psQ = ctx.enter_context(tc.tile_pool(name="psQ", bufs=1, space="PSUM"))
    psV = ctx.enter_context(tc.tile_pool(name="psV", bufs=2, space="PSUM"))
    psA = ctx.enter_context(tc.tile_pool(name="psA", bufs=1, space="PSUM"))
    psD = ctx.enter_context(tc.tile_pool(name="psD", bufs=1, space="PSUM"))
    psO = ctx.enter_context(tc.tile_pool(name="psO", bufs=1, space="PSUM"))

    # ---- constants -------------------------------------------------------
    wqk_sb = const.tile([128, 2, 128], F16)
    nc.sync.dma_start(out=wqk_sb, in_=wqk_d[:, :].rearrange("(kc p) m -> p kc m", p=128))
    wv_sb = const.tile([128, 2, C], F16)
    nc.sync.dma_start(out=wv_sb, in_=wv_d[:, :].rearrange("(kc p) m -> p kc m", p=128))
    w1_sb = const.tile([128, 2, HID], F32)
    nc.sync.dma_start(out=w1_sb, in_=w1_d[:, :].rearrange("(kc p) m -> p kc m", p=128))
    w2_sb = const.tile([HID, 2, 128], F32)
    nc.sync.dma_start(out=w2_sb, in_=w2_d[:, :].rearrange("k (mc m) -> k mc m", m=128))
    gama_sb = const.tile([128, 1], F32)
    nc.sync.dma_start(out=gama_sb, in_=gama_d[:, :].to_broadcast([128, 1]))
    ones_col = const.tile([128, 1], BF16)
    nc.vector.memset(ones_col, 1.0)
    gscale = const.tile([128, 2], F32)      # gama * sigmoid(gate), filled later

    sums_acc = stats.tile([128, 2, NBLK], F32)
    nc.vector.memset(sums_acc, 0.0)
    # ping-pong max accumulators (plain TensorTensor max runs in DVE 2x mode;
    # tensor_reduce measured 4x slower per element)
    acc_a = stats.tile([128, 2, RB, W], F16)
    nc.vector.memset(acc_a, -60000.0)
    acc_b = stats.tile([128, 2, RB, W], F16)

    # ---- resident x: all DMAs issued up front ---------------------------
    x_tiles = []
    for b in range(NBLK):
        h0 = b * RB
        xb = xpool.tile([128, 2, RB, W], F16, tag=f"x{b}")
        nc.sync.dma_start(
            out=xb,
            in_=xh_d[:, h0:h0 + RB, :].rearrange("(kc p) h w -> p kc h w", p=128),
        )
        x_tiles.append(xb)

    # single big resident attention-output tile (fine-grained dep tracking is
    # not needed: pass-2 and the max reduces only read after prior writes)
    ob_all = opool.tile([128, 2, NBLK, RB, W], F16, tag="ob")

    # ---- pass 1 ----------------------------------------------------------
    # The q/k projection for block b+1 runs one iteration ahead so the
    # qk-copy -> qq-DMA chain has a whole block of slack before the attT
    # matmuls consume it: the PE never waits on the SBUF->SBUF replicate.
    def project_qk(b):
        xb = x_tiles[b]
        qk_ps = psQ.tile([128, RB, W], F32, tag="qk")
        for kc in (0, 1):
            nc.tensor.matmul(
                out=qk_ps.rearrange("p r w -> p (r w)"),
                lhsT=wqk_sb[:, kc, :],
                rhs=xb[:, kc, :, :].rearrange("p r w -> p (r w)"),
                start=(kc == 0), stop=(kc == 1),
            )
        qk_sb = work.tile([128, RB, W], F16, tag="qk_sb")
        nc.scalar.copy(out=qk_sb, in_=qk_ps)
        # replicate q down to base partition 0 (matmul operands share base);
        # issue from the Scalar engine so the SP DMA queue never stalls on it
        qq_sb = work.tile([64, RB, W], F16, tag="qq_sb")
        nc.scalar.dma_start(out=qq_sb, in_=qk_sb[64:128, :, :])
        return qk_sb, qq_sb

    qk_next = project_qk(0)
    for b in range(NBLK):
        xb = x_tiles[b]
        qk_sb, qq_sb = qk_next
        if b + 1 < NBLK:
            qk_next = project_qk(b + 1)

        # v^T per row (w on partitions, c on free)
        vt_sb = work.tile([128, RB, C], BF16, tag="vt_sb")
        for rp in range(RB // 2):
            vt_ps = psV.tile([128, 2, C], F32, tag="vt")
            for rr in (0, 1):
                r = rp * 2 + rr
                for kc in (0, 1):
                    nc.tensor.matmul(
                        out=vt_ps[:, rr, :],
                        lhsT=xb[:, kc, r, :],
                        rhs=wv_sb[:, kc, :],
                        start=(kc == 0), stop=(kc == 1),
                    )
            nc.scalar.copy(out=vt_sb[:, rp * 2:rp * 2 + 2, :], in_=vt_ps)

        # transposed attention: attT[j, i] = sum_d k[d, j] q[d, i]
        attT_ps = psA.tile([128, RB, W], F32, tag="attT")
        for r in range(RB):
            nc.tensor.matmul(
                out=attT_ps[:, r, :],
                lhsT=qk_sb[0:64, r, :],
                rhs=qq_sb[:, r, :],
                start=True, stop=True,
            )
        # batched exp for the whole block; no max-subtraction (|score|<~40)
        att_e = work.tile([128, RB, W], BF16, tag="att_e")
        nc.scalar.activation(out=att_e, in_=attT_ps, func=AF.Exp)

        # denominator via ones-matmul (cross-partition sum), then 1/x on DVE
        den_ps = psD.tile([1, RB, W], F32, tag="den")
        nc.tensor.matmul(
            out=den_ps.rearrange("p r w -> p (r w)"),
            lhsT=ones_col,
            rhs=att_e.rearrange("p r w -> p (r w)"),
            start=True, stop=True,
        )
        inv_sb = work.tile([1, RB, W], F32, tag="inv")
        # ~18-bit 1/x, ~5x faster than the exact InstReciprocal (which
        # measured 3.3us per 512-element call on hardware)
        nc.vector.reciprocal_approx_fast(out=inv_sb, in_=den_ps)
        # broadcast 1/den across partitions on GPSIMD (SBUF->SBUF)
        binv_sb = work.tile([128, RB, W], F32, tag="binv")
        nc.gpsimd.partition_broadcast(binv_sb[:, :, :], inv_sb[:, :, :])

        # unnormalized out; normalization + mean stat ride the PSUM->SBUF copy
        out_ps = psO.tile([128, 2, RB, W], F32, tag="out")
        for r in range(RB):
            for ch in (0, 1):
                nc.tensor.matmul(
                    out=out_ps[:, ch, r, :],
                    lhsT=vt_sb[:, r, 128 * ch:128 * (ch + 1)],
                    rhs=att_e[:, r, :],
                    start=True, stop=True,
                )

        for ch in (0, 1):
            nc.vector.scalar_tensor_tensor(
                out=ob_all[:, ch, b], in0=out_ps[:, ch], scalar=1.0,
                in1=binv_sb, op0=OP.mult, op1=OP.mult,
                accum_out=sums_acc[:, ch, b:b + 1])
        # running max stat (DVE TensorTensor, fp16 2x mode), ping-pong accs
        src, dst = (acc_a, acc_b) if b % 2 == 0 else (acc_b, acc_a)
        nc.vector.tensor_tensor(out=dst, in0=src, in1=ob_all[:, :, b],
                                op=OP.max)

    # ---- gate ------------------------------------------------------------
    sums = stats.tile([128, 2], F32)
    nc.vector.tensor_reduce(out=sums, in_=sums_acc, axis=AX.X, op=OP.add)

    mx = stats.tile([128, 2], F32)
    final_acc = acc_a if NBLK % 2 == 0 else acc_b
    nc.vector.tensor_reduce(out=mx, in_=final_acc, axis=AX.XY, op=OP.max)

    mlp_in = stats.tile([128, 2, 2], F32)
    nc.vector.tensor_scalar_mul(out=mlp_in[:, :, 0], in0=sums, scalar1=INV_HW)
    nc.vector.tensor_copy(out=mlp_in[:, :, 1], in_=mx)

    h_ps = psD.tile([HID, 2], F32, tag="den")
    for kc in (0, 1):
        nc.tensor.matmul(
            out=h_ps,
            lhsT=w1_sb[:, kc, :],
            rhs=mlp_in[:, kc, :],
            start=(kc == 0), stop=(kc == 1),
        )
    hr = stats.tile([HID, 2], F32)
    nc.vector.tensor_scalar_max(out=hr, in0=h_ps, scalar1=0.0)
    g_ps = psQ.tile([128, 2, 2], F32, tag="qk")
    for mc in (0, 1):
        nc.tensor.matmul(
            out=g_ps[:, mc, :],
            lhsT=w2_sb[:, mc, :],
            rhs=hr,
            start=True, stop=True,
        )
    zt = stats.tile([128, 2], F32)
    nc.vector.tensor_reduce(out=zt, in_=g_ps, axis=AX.X, op=OP.add)
    th = stats.tile([128, 2], F32)
    nc.scalar.activation(out=th, in_=zt, func=AF.Tanh, scale=0.5)
    u = stats.tile([128, 2], F32)
    nc.vector.tensor_scalar_add(out=u, in0=th, scalar1=1.0)
    # gscale = gama * sigmoid(z) = gama * 0.5 * (1 + tanh(z/2))
    nc.vector.tensor_scalar(
        out=gscale, in0=u, scalar1=gama_sb, scalar2=0.5, op0=OP.mult, op1=OP.mult)

    # ---- pass 2: final = out*gscale[c] + x ------------------------------
    for b in range(NBLK):
        h0 = b * RB
        xb = x_tiles[b]
        fin = finpool.tile([128, 2, RB, W], F16, tag="fin")
        for kc in (0, 1):
            nc.vector.scalar_tensor_tensor(
                out=fin[:, kc], in0=ob_all[:, kc, b],
                scalar=gscale[:, kc:kc + 1],
                in1=xb[:, kc], op0=OP.mult, op1=OP.add)
        nc.sync.dma_start(
            out=y_d[:, h0:h0 + RB, :].rearrange("(kc p) h w -> p kc h w", p=128),
            in_=fin,
        )


def build_nc() -> bass.Bass:
    nc = bacc.Bacc()
    xh_d = nc.dram_tensor("xh", [C, H, W], F16, kind="ExternalInput")
    wqk_d = nc.dram_tensor("wqkT", [C, 128], F16, kind="ExternalInput")
    wv_d = nc.dram_tensor("wvT", [C, C], F16, kind="ExternalInput")
    w1_d = nc.dram_tensor("w1T", [C, HID], F32, kind="ExternalInput")
    w2_d = nc.dram_tensor("w2T", [HID, C], F32, kind="ExternalInput")
    gama_d = nc.dram_tensor("gama", [1, 1], F32, kind="ExternalInput")
    y_d = nc.dram_tensor("out", [C, H, W], F16, kind="ExternalOutput")

    with tile.TileContext(nc) as tc:
        with ExitStack() as ctx:
            _body(ctx, tc, xh_d[:, :, :], wqk_d[:, :],
                  wv_d[:, :], w1_d[:, :], w2_d[:, :], gama_d[:, :],
                  y_d[:, :, :])
    nc.compile()
    return nc


_NC_CACHE = {}


def _get_nc():
    if "nc" not in _NC_CACHE:
        _NC_CACHE["nc"] = build_nc()
    return _NC_CACHE["nc"]


def _make_in_maps(x, Wq, Wk, Wv, W1, W2, gama):
    wqkT = np.ascontiguousarray(
        np.concatenate([Wk, Wq], axis=0).T.astype(np.float16))
    wvT = np.ascontiguousarray(Wv.T.astype(np.float16))
    w1T = np.ascontiguousarray(W1.T.astype(np.float32))
    w2T = np.ascontiguousarray(W2.T.astype(np.float32))
    g = np.asarray(gama, dtype=np.float32).reshape(1, 1)
    maps = []
    for i in range(NCORES):
        maps.append({
            "xh": np.ascontiguousarray(x[i].astype(np.float16)),
            "wqkT": wqkT, "wvT": wvT, "w1T": w1T, "w2T": w2T, "gama": g,
        })
    return maps


def run(x, Wq, Wk, Wv, W1, W2, gama, trace=False):
    nc = _get_nc()
    in_maps = _make_in_maps(x, Wq, Wk, Wv, W1, W2, gama)
    res = run_bass_kernel_spmd(nc, in_maps, core_ids=list(range(NCORES)),
                               trace=trace)
    y = np.stack([res.results[i]["out"] for i in range(NCORES)], axis=0)
    return y, res


def kernel(x, Wq, Wk, Wv, W1, W2, gama):
    x = np.asarray(x); Wq = np.asarray(Wq); Wk = np.asarray(Wk)
    Wv = np.asarray(Wv); W1 = np.asarray(W1); W2 = np.asarray(W2)
    gama = np.asarray(gama)
    y, _ = run(x, Wq, Wk, Wv, W1, W2, gama, trace=False)
    return y.astype(np.float32)


# revision 25
# speedup vs baseline: 1.5153x; 1.0015x over previous
"""ChannelRowAttention Trainium2 kernel (v2).

Full-input contract: kernel(**inputs) takes the complete (8,256,128,128) batch
plus weights, shards batch-wise across 8 NeuronCores (one image per core), and
returns the full (8,256,128,128) output.

v2 design (per core, x_img = (256,128,128)):
  HBM traffic minimized to x-in fp16 (16.8MB) + y-out fp16 (16.8MB); x stays
  resident in SBUF for both passes, attention output stays resident fp16.

  pass 1, per 4-row block:
    qk    = [Wk|Wq]^T . x_rows       one fused M=128 matmul pair (k on
            partitions 0:64, q on 64:128); ACT copy to fp16; q replicated to
            a base-0 tile via SBUF->SBUF DMA
    attT  = k^T q  (PE, K=64)        -- attention computed TRANSPOSED, so no
            PE transpose and one batched exp serves the whole block
    att_e = exp(attT) -> bf16        (one ACT instr per block)
    den   = ones^T . att_e           (PE matmul, M=1)
    inv   = 1/den                    (DVE reciprocal, fp32)
    binv  = broadcast(inv) across partitions via SBUF->SBUF DMA (free)
    vT    = x_row^T . Wv^T           (PE, per row; copies split ACT/GPSIMD)
    out_u = vT^T . att_e             (PE; unnormalized)
    out   = out_u * binv             rides the PSUM->SBUF copy (DVE STT);
            per-channel mean rides accum_out; running max on GPSIMD
  gate  = sigmoid(W2.relu(W1.avg) + W2.relu(W1.max)) via tanh trick
  pass 2, per block: final = out*(gama*gate[c]) + x -> fp16 -> DRAM
"""

import numpy as np
from contextlib import ExitStack

import concourse.bass as bass
from concourse import bacc
import concourse.tile as tile
from concourse import mybir
from concourse.bass_utils import run_bass_kernel_spmd

F32 = mybir.dt.float32
F16 = mybir.dt.float16
BF16 = mybir.dt.bfloat16

N, C, H, W = 8, 256, 128, 128
QK = 64
HID = 16          # SE hidden dim = C // 16
NCORES = 8
RB = 4            # rows per block
NBLK = H // RB    # 32
INV_HW = 1.0 / float(H * W)

AX = mybir.AxisListType
OP = mybir.AluOpType
AF = mybir.ActivationFunctionType


def _body(ctx: ExitStack, tc: "tile.TileContext", xh_d, wqk_d, wv_d,
          w1_d, w2_d, gama_d, y_d):
    nc = tc.nc

    const = ctx.enter_context(tc.tile_pool(name="const", bufs=1))
    stats = ctx.enter_context(tc.tile_pool(name="stats", bufs=1))
    xpool = ctx.enter_context(tc.tile_pool(name="xp", bufs=1))
    opool = ctx.enter_context(tc.tile_pool(name="op", bufs=1))
    work = ctx.enter_context(tc.tile_pool(name="work", bufs=3))
    finpool = ctx.enter_context(tc.tile_pool(name="fin", bufs=3))
    psQ = ctx.enter_context(tc.tile_pool(name="psQ", bufs=1, space="PSUM"))
    psV = ctx.enter_context(tc.tile_pool(name="psV", bufs=1, space="PSUM"))
    psA = ctx.enter_context(tc.tile_pool(name="psA", bufs=1, space="PSUM"))
    psD = ctx.enter_context(tc.tile_pool(name="psD", bufs=1, space="PSUM"))
    # 2 bufs so out-mm(b+1) never waits for norm(b) to drain the bank — the
    # recip->broadcast->norm chain is the longest per-block dependency cycle
    psO = ctx.enter_context(tc.tile_pool(name="psO", bufs=2, space="PSUM"))

    # ---- constants -------------------------------------------------------
    wqk_sb = const.tile([128, 2, 128], F16)
    nc.sync.dma_start(out=wqk_sb, in_=wqk_d[:, :].rearrange("(kc p) m -> p kc m", p=128))
    wv_sb = const.tile([128, 2, C], F16)
    nc.sync.dma_start(out=wv_sb, in_=wv_d[:, :].rearrange("(kc p) m -> p kc m", p=128))
    w1_sb = const.tile([128, 2, HID], F32)
    nc.sync.dma_start(out=w1_sb, in_=w1_d[:, :].rearrange("(kc p) m -> p kc m", p=128))
    w2_sb = const.tile([HID, 2, 128], F32)
    nc.sync.dma_start(out=w2_sb, in_=w2_d[:, :].rearrange("k (mc m) -> k mc m", m=128))
    gama_sb = const.tile([128, 1], F32)
    nc.sync.dma_start(out=gama_sb, in_=gama_d[:, :].to_broadcast([128, 1]))
    ones_col = const.tile([128, 1], BF16)
    nc.vector.memset(ones_col, 1.0)
    gscale = const.tile([128, 2], F32)      # gama * sigmoid(gate), filled later

    sums_acc = stats.tile([128, 2, NBLK], F32)
    nc.vector.memset(sums_acc, 0.0)
    # ping-pong max accumulators (plain TensorTensor max runs in DVE 2x mode;
    # tensor_reduce measured 4x slower per element)
    acc_a = stats.tile([128, 2, RB, W], F16)
    nc.vector.memset(acc_a, -60000.0)
    acc_b = stats.tile([128, 2, RB, W], F16)

    # ---- resident x: all DMAs issued up front ---------------------------
    x_tiles = []
    for b in range(NBLK):
        h0 = b * RB
        xb = xpool.tile([128, 2, RB, W], F16, tag=f"x{b}")
        nc.sync.dma_start(
            out=xb,
            in_=xh_d[:, h0:h0 + RB, :].rearrange("(kc p) h w -> p kc h w", p=128),
        )
        x_tiles.append(xb)

    # single big resident attention-output tile (fine-grained dep tracking is
    # not needed: pass-2 and the max reduces only read after prior writes)
    ob_all = opool.tile([128, 2, NBLK, RB, W], F16, tag="ob")

    # ---- pass 1 ----------------------------------------------------------
    # The q/k projection for block b+1 runs one iteration ahead so the
    # qk-copy -> qq-DMA chain has a whole block of slack before the attT
    # matmuls consume it: the PE never waits on the SBUF->SBUF replicate.
    def project_qk(b):
        xb = x_tiles[b]
        qk_ps = psQ.tile([128, RB, W], F32, tag="qk")
        for kc in (0, 1):
            nc.tensor.matmul(
                out=qk_ps.rearrange("p r w -> p (r w)"),
                lhsT=wqk_sb[:, kc, :],
                rhs=xb[:, kc, :, :].rearrange("p r w -> p (r w)"),
                start=(kc == 0), stop=(kc == 1),
            )
        qk_sb = work.tile([128, RB, W], F16, tag="qk_sb")
        nc.scalar.copy(out=qk_sb, in_=qk_ps)
        # replicate q down to base partition 0 (matmul operands share base);
        # issue from the Scalar engine so the SP DMA queue never stalls on it
        qq_sb = work.tile([64, RB, W], F16, tag="qq_sb")
        nc.scalar.dma_start(out=qq_sb, in_=qk_sb[64:128, :, :])
        return qk_sb, qq_sb

    qk_next = project_qk(0)
    for b in range(NBLK):
        xb = x_tiles[b]
        qk_sb, qq_sb = qk_next
        if b + 1 < NBLK:
            qk_next = project_qk(b + 1)

        # v^T per row (w on partitions, c on free)
        vt_sb = work.tile([128, RB, C], BF16, tag="vt_sb")
        for rp in range(RB // 2):
            vt_ps = psV.tile([128, 2, C], F32, tag="vt")
            for rr in (0, 1):
                r = rp * 2 + rr
                for kc in (0, 1):
                    nc.tensor.matmul(
                        out=vt_ps[:, rr, :],
                        lhsT=xb[:, kc, r, :],
                        rhs=wv_sb[:, kc, :],
                        start=(kc == 0), stop=(kc == 1),
                    )
            nc.scalar.copy(out=vt_sb[:, rp * 2:rp * 2 + 2, :], in_=vt_ps)

        # transposed attention: attT[j, i] = sum_d k[d, j] q[d, i]
        attT_ps = psA.tile([128, RB, W], F32, tag="attT")
        for r in range(RB):
            nc.tensor.matmul(
                out=attT_ps[:, r, :],
                lhsT=qk_sb[0:64, r, :],
                rhs=qq_sb[:, r, :],
                start=True, stop=True,
            )
        # batched exp for the whole block; no max-subtraction (|score|<~40)
        att_e = work.tile([128, RB, W], BF16, tag="att_e")
        nc.scalar.activation(out=att_e, in_=attT_ps, func=AF.Exp)

        # denominator via ones-matmul (cross-partition sum), then 1/x on DVE
        den_ps = psD.tile([1, RB, W], F32, tag="den")
        nc.tensor.matmul(
            out=den_ps.rearrange("p r w -> p (r w)"),
            lhsT=ones_col,
            rhs=att_e.rearrange("p r w -> p (r w)"),
            start=True, stop=True,
        )
        inv_sb = work.tile([1, RB, W], F32, tag="inv")
        # ~18-bit 1/x, ~5x faster than the exact InstReciprocal (which
        # measured 3.3us per 512-element call on hardware)
        nc.vector.reciprocal_approx_fast(out=inv_sb, in_=den_ps)
        # broadcast 1/den across partitions on GPSIMD (SBUF->SBUF)
        binv_sb = work.tile([128, RB, W], F32, tag="binv")
        nc.gpsimd.partition_broadcast(binv_sb[:, :, :], inv_sb[:, :, :])

        # unnormalized out; normalization + mean stat ride the PSUM->SBUF copy
        out_ps = psO.tile([128, 2, RB, W], F32, tag="out")
        for r in range(RB):
            for ch in (0, 1):
                nc.tensor.matmul(
                    out=out_ps[:, ch, r, :],
                    lhsT=vt_sb[:, r, 128 * ch:128 * (ch + 1)],
                    rhs=att_e[:, r, :],
                    start=True, stop=True,
                )

        for ch in (0, 1):
            nc.vector.scalar_tensor_tensor(
                out=ob_all[:, ch, b], in0=out_ps[:, ch], scalar=1.0,
                in1=binv_sb, op0=OP.mult, op1=OP.mult,
                accum_out=sums_acc[:, ch, b:b + 1])
        # running max stat (DVE TensorTensor, fp16 2x mode), ping-pong accs
        src, dst = (acc_a, acc_b) if b % 2 == 0 else (acc_b, acc_a)
        nc.vector.tensor_tensor(out=dst, in0=src, in1=ob_all[:, :, b],
                                op=OP.max)

    # ---- gate ------------------------------------------------------------
    sums = stats.tile([128, 2], F32)
    nc.vector.tensor_reduce(out=sums, in_=sums_acc, axis=AX.X, op=OP.add)

    mx = stats.tile([128, 2], F32)
    final_acc = acc_a if NBLK % 2 == 0 else acc_b
    nc.vector.tensor_reduce(out=mx, in_=final_acc, axis=AX.XY, op=OP.max)

    mlp_in = stats.tile([128, 2, 2], F32)
    nc.vector.tensor_scalar_mul(out=mlp_in[:, :, 0], in0=sums, scalar1=INV_HW)
    nc.vector.tensor_copy(out=mlp_in[:, :, 1], in_=mx)

    h_ps = psD.tile([HID, 2], F32, tag="den")
    for kc in (0, 1):
        nc.tensor.matmul(
            out=h_ps,
            lhsT=w1_sb[:, kc, :],
            rhs=mlp_in[:, kc, :],
            start=(kc == 0), stop=(kc == 1),
        )
    hr = stats.tile([HID, 2], F32)
    nc.vector.tensor_scalar_max(out=hr, in0=h_ps, scalar1=0.0)
    g_ps = psQ.tile([128, 2, 2], F32, tag="qk")
    for mc in (0, 1):
        nc.tensor.matmul(
            out=g_ps[:, mc, :],
            lhsT=w2_sb[:, mc, :],
            rhs=hr,
            start=True, stop=True,
        )
    zt = stats.tile([128, 2], F32)
    nc.vector.tensor_reduce(out=zt, in_=g_ps, axis=AX.X, op=OP.add)
    th = stats.tile([128, 2], F32)
    nc.scalar.activation(out=th, in_=zt, func=AF.Tanh, scale=0.5)
    u = stats.tile([128, 2], F32)
    nc.vector.tensor_scalar_add(out=u, in0=th, scalar1=1.0)
    # gscale = gama * sigmoid(z) = gama * 0.5 * (1 + tanh(z/2))
    nc.vector.tensor_scalar(
        out=gscale, in0=u, scalar1=gama_sb, scalar2=0.5, op0=OP.mult, op1=OP.mult)

    # ---- pass 2: final = out*gscale[c] + x ------------------------------
    for b in range(NBLK):
        h0 = b * RB
        xb = x_tiles[b]
        fin = finpool.tile([128, 2, RB, W], F16, tag="fin")
        for kc in (0, 1):
            nc.vector.scalar_tensor_tensor(
                out=fin[:, kc], in0=ob_all[:, kc, b],
                scalar=gscale[:, kc:kc + 1],
                in1=xb[:, kc], op0=OP.mult, op1=OP.add)
        nc.sync.dma_start(
            out=y_d[:, h0:h0 + RB, :].rearrange("(kc p) h w -> p kc h w", p=128),
            in_=fin,
        )


def build_nc() -> bass.Bass:
    nc = bacc.Bacc()
    xh_d = nc.dram_tensor("xh", [C, H, W], F16, kind="ExternalInput")
    wqk_d = nc.dram_tensor("wqkT", [C, 128], F16, kind="ExternalInput")
    wv_d = nc.dram_tensor("wvT", [C, C], F16, kind="ExternalInput")
    w1_d = nc.dram_tensor("w1T", [C, HID], F32, kind="ExternalInput")
    w2_d = nc.dram_tensor("w2T", [HID, C], F32, kind="ExternalInput")
    gama_d = nc.dram_tensor("gama", [1, 1], F32, kind="ExternalInput")
    y_d = nc.dram_tensor("out", [C, H, W], F16, kind="ExternalOutput")

    with tile.TileContext(nc) as tc:
        with ExitStack() as ctx:
            _body(ctx, tc, xh_d[:, :, :], wqk_d[:, :],
                  wv_d[:, :], w1_d[:, :], w2_d[:, :], gama_d[:, :],
                  y_d[:, :, :])
    nc.compile()
    return nc


_NC_CACHE = {}


def _get_nc():
    if "nc" not in _NC_CACHE:
        _NC_CACHE["nc"] = build_nc()
    return _NC_CACHE["nc"]


def _make_in_maps(x, Wq, Wk, Wv, W1, W2, gama):
    wqkT = np.ascontiguousarray(
        np.concatenate([Wk, Wq], axis=0).T.astype(np.float16))
    wvT = np.ascontiguousarray(Wv.T.astype(np.float16))
    w1T = np.ascontiguousarray(W1.T.astype(np.float32))
    w2T = np.ascontiguousarray(W2.T.astype(np.float32))
    g = np.asarray(gama, dtype=np.float32).reshape(1, 1)
    maps = []
    for i in range(NCORES):
        maps.append({
            "xh": np.ascontiguousarray(x[i].astype(np.float16)),
            "wqkT": wqkT, "wvT": wvT, "w1T": w1T, "w2T": w2T, "gama": g,
        })
    return maps


def run(x, Wq, Wk, Wv, W1, W2, gama, trace=False):
    nc = _get_nc()
    in_maps = _make_in_maps(x, Wq, Wk, Wv, W1, W2, gama)
    res = run_bass_kernel_spmd(nc, in_maps, core_ids=list(range(NCORES)),
                               trace=trace)
    y = np.stack([res.results[i]["out"] for i in range(NCORES)], axis=0)
    return y, res


def kernel(x, Wq, Wk, Wv, W1, W2, gama):
    x = np.asarray(x); Wq = np.asarray(Wq); Wk = np.asarray(Wk)
    Wv = np.asarray(Wv); W1 = np.asarray(W1); W2 = np.asarray(W2)
    gama = np.asarray(gama)
    y, _ = run(x, Wq, Wk, Wv, W1, W2, gama, trace=False)
    return y.astype(np.float32)
